# revision 26
# baseline (speedup 1.0000x reference)
"""AxialRoPE self-attention on 8 Trainium2 NeuronCores.

Sharding: 8 cores = 4 batches x 2 head-groups (8 heads each).
Each core computes q/k/v projections for its head-group over the full
sequence of its batch, RoPE, attention, and a partial output projection
(row-sharded Wo). Host sums the two partial outputs per batch.

Per-core kernel (all matmuls bf16 with fp32 PSUM accumulation):
  xT [1024, 2048] = x[b]^T (host-prepped bf16)
  KT/QT = W^T x^T + b   [512, 2048] head-dim-major (K emitted first so
  attention unblocks sooner).
  RoPE: qt' = QT*cosT + shift(QT)*sinTs where shift swaps partition pairs
  (2i <-> 2i+1) via two partition-strided SBUF DMAs and sinTs carries the
  (-1)^(d+1) sign.
  scoresT[ks, qs]: the two heads of a pair land in adjacent row groups
  (lhsT base partitions 0/64) so their matmuls overlap in the PE array;
  both halves fill one [128, 1024] psum tile -> single exp (scale=1/8,
  no max subtraction: scores ~N(0,1), max < 7).
  PV: lhsT = V_aug [ks, 65] (65th column of ones -> row 64 = softmax
  denominator). Attention outputs accumulate UNNORMALIZED into
  aoh2[hp] [128, 2048] (head pair packed on partitions); denominators
  collect into densb [8, 2048].
  While attention for pair dc runs (ACT-bound), the Q/K projection
  matmuls for pair dc+1 are interleaved into the instruction stream as
  fillers for the PE gaps.
  Normalization is deferred to one block after all attention: one
  Ln + one Exp(-x) over densb (single act-table switch), then per
  (hp, q-tile) a K=2 selector matmul broadcasts the two reciprocal rows
  across the 128 partitions and one DVE multiply rescales aoh2 in place.
  Output projection: lhsT = aoh2[hp][:, sc] [128, 128] (K=128, both
  heads of the pair in one matmul), rhs = wo[hp*128:, :].
"""

import os
import numpy as np

B, S, D = 4, 2048, 1024
NHEAD, HDIM = 16, 64
HG = 2                # head-group shards
HPC = NHEAD // HG     # 8 heads per core
DG = HPC * HDIM       # 512 local projection width
NCORES = 8
ROPE_BASE = 10000.0

_CACHE = {}


def _build_program():
    from concourse import bass, bacc, tile
    from concourse import mybir

    dt = mybir.dt
    f32, bf16 = dt.float32, dt.bfloat16
    AF = mybir.ActivationFunctionType
    ALU = mybir.AluOpType
    PSUM = bass.MemorySpace.PSUM

    nc = bacc.Bacc("TRN2", target_bir_lowering=False, debug=False)

    # The PJRT-side NEFF cache keys on the HLO signature, which sees only
    # tensor shapes -- encode a build nonce in a dummy input's shape so
    # program variants with identical I/O still recompile.
    _nw = (int(os.environ.get("BUILD_REPEAT", "1"))
           + 100 * int(os.environ.get("BUILD_NONCE", "0")))
    nc.dram_tensor("nonce", [1, _nw], f32, kind="ExternalInput")

    xT_d = nc.dram_tensor("xT", [D, S], bf16, kind="ExternalInput")
    wq_d = nc.dram_tensor("wq", [D, DG], bf16, kind="ExternalInput")
    wk_d = nc.dram_tensor("wk", [D, DG], bf16, kind="ExternalInput")
    wv_d = nc.dram_tensor("wv", [D, DG], bf16, kind="ExternalInput")
    wo_d = nc.dram_tensor("wo", [DG, D], bf16, kind="ExternalInput")
    cos_d = nc.dram_tensor("cosT", [128, S], bf16, kind="ExternalInput")
    sin_d = nc.dram_tensor("sinTs", [128, S], bf16, kind="ExternalInput")
    bq_d = nc.dram_tensor("bq4", [128, 4], f32, kind="ExternalInput")
    bk_d = nc.dram_tensor("bk4", [128, 4], f32, kind="ExternalInput")
    bv_d = nc.dram_tensor("bv", [1, DG], bf16, kind="ExternalInput")
    bo_d = nc.dram_tensor("bo", [1, D], bf16, kind="ExternalInput")
    sel_d = nc.dram_tensor("sel", [2, 128], bf16, kind="ExternalInput")
    out_d = nc.dram_tensor("out", [S, D], f32, kind="ExternalOutput")

    CC = D // 128    # 8 contraction chunks
    DC = DG // 128   # 4 dout chunks (2 heads each)
    SC = S // 128    # 16 sequence chunks
    QT2 = 512        # query tile
    NQ = S // QT2    # 4

    with tile.TileContext(nc) as tc:
        with (
            tc.tile_pool(name="persist", bufs=1) as P,
            tc.tile_pool(name="ps_all", bufs=2, space=PSUM) as PS,
            tc.tile_pool(name="tmp", bufs=2) as T1,
            tc.tile_pool(name="wstream", bufs=16) as WS,
            tc.tile_pool(name="wvp", bufs=8) as WV,
            tc.tile_pool(name="ptp", bufs=2) as PT,
        ):
            _REPEAT = int(os.environ.get("BUILD_REPEAT", "1"))
            for _rep in range(_REPEAT):
                # ---- weight DMAs first (small, unblock projections) ----
                wqk_sb = {}
                for wi, w_d in enumerate([wk_d, wq_d]):  # K first
                    wqk_sb[wi] = [WS.tile([128, DG], bf16, tag="w", name=f"w{wi}_{_}")
                                  for _ in range(CC)]
                    for i in range(CC):
                        nc.sync.dma_start(wqk_sb[wi][i][:], w_d.ap()[i * 128:(i + 1) * 128, :])
                xt = [P.tile([128, S], bf16, tag=f"xt{i}", name=f"xt{i}") for i in range(CC)]
                for i in range(CC):
                    # gpsimd/act queues: overlap with the weight DMAs on sync
                    eng = nc.gpsimd if i % 2 == 0 else nc.scalar
                    eng.dma_start(xt[i][:], xT_d.ap()[i * 128:(i + 1) * 128, :])
                cos_t = P.tile([128, S], bf16, tag="cos")
                sin_t = P.tile([128, S], bf16, tag="sin")
                nc.sync.dma_start(cos_t[:], cos_d.ap()[:])
                nc.sync.dma_start(sin_t[:], sin_d.ap()[:])
                bq4 = P.tile([128, 4], f32, tag="bq4")
                bk4 = P.tile([128, 4], f32, tag="bk4")
                bv_sb = P.tile([1, DG], bf16, tag="bv_sb")
                nc.sync.dma_start(bq4[:], bq_d.ap()[:])
                nc.sync.dma_start(bk4[:], bk_d.ap()[:])
                nc.sync.dma_start(bv_sb[:], bv_d.ap()[:])

                ones = P.tile([1, 128], bf16, tag="ones")
                nc.vector.memset(ones[:], 1.0)
                # selector for denominator broadcast: out row j gets
                # recip row 0 (j<64) or row 1 (j>=64)
                sel = P.tile([2, 128], bf16, tag="sel")
                nc.sync.dma_start(sel[:], sel_d.ap()[:])

                qt = [P.tile([128, S], bf16, tag=f"qt{i}", name=f"qt{i}") for i in range(DC)]
                kt = [P.tile([128, S], bf16, tag=f"kt{i}", name=f"kt{i}") for i in range(DC)]
                vaug = [P.tile([128, HPC * 65], bf16, tag=f"va{i}", name=f"va{i}") for i in range(SC)]
                aoh2 = [P.tile([128, S], bf16, tag=f"ao{i}", name=f"ao{i}") for i in range(DC)]
                densb = P.tile([8, S], f32, tag="densb")
                densbr = P.tile([8, S], f32, tag="densbr")
                densbb = P.tile([8, S], bf16, tag="densbb")

                # ---- Q/K projection + rope for pair dc, as a list of
                # filler closures (each emits ~1 PE matmul or the DVE/DMA
                # epilogue for one 512-col s-tile) ----
                def qk_fillers(dc):
                    dsl = slice(dc * 128, (dc + 1) * 128)
                    fillers = []
                    state = {}

                    def mk_mm(wi, st, cc):
                        def emit():
                            sl = slice(st * 512, (st + 1) * 512)
                            if cc == 0:
                                state[(wi, st)] = PS.tile(
                                    [128, 512], f32, tag="proj", name=f"psp{dc}_{wi}_{st}")
                            ps = state[(wi, st)]
                            nc.tensor.matmul(
                                ps[:], wqk_sb[wi][cc][:, dsl], xt[cc][:, sl],
                                start=(cc == 0), stop=(cc == CC - 1),
                            )
                        return emit

                    def mk_bias(wi, st):
                        def emit():
                            sl = slice(st * 512, (st + 1) * 512)
                            b4 = bk4 if wi == 0 else bq4
                            qtsb = state[("sb", wi)]
                            nc.vector.tensor_scalar(
                                qtsb[:, sl], state[(wi, st)][:], b4[:, dc:dc + 1],
                                None, op0=ALU.add,
                            )
                        return emit

                    def mk_sb(wi):
                        def emit():
                            state[("sb", wi)] = T1.tile(
                                [128, S], bf16, tag="qtsb", bufs=2, name=f"qtsb{dc}_{wi}")
                        return emit

                    def mk_rope(wi):
                        def emit():
                            dst = (kt if wi == 0 else qt)[dc]
                            qtsb = state[("sb", wi)]
                            qsh = T1.tile([128, S], bf16, tag="qsh", bufs=1)
                            for blk in range(2):
                                b0 = 64 * blk
                                nc.sync.dma_start(
                                    qsh[b0:b0 + 32, :], qtsb[b0 + 32:b0 + 64, :])
                                nc.sync.dma_start(
                                    qsh[b0 + 32:b0 + 64, :], qtsb[b0:b0 + 32, :])
                            nc.vector.tensor_tensor(dst[:], qtsb[:], cos_t[:], op=ALU.mult)
                            tt2 = T1.tile([128, S], bf16, tag="tt2", bufs=1)
                            nc.vector.tensor_tensor(tt2[:], qsh[:], sin_t[:], op=ALU.mult)
                            nc.vector.tensor_tensor(dst[:], dst[:], tt2[:], op=ALU.add)
                        return emit

                    for wi in range(2):  # 0=K, 1=Q
                        fillers.append(mk_sb(wi))
                        for st in range(4):
                            for cc in range(CC):
                                fillers.append(mk_mm(wi, st, cc))
                            fillers.append(mk_bias(wi, st))
                        fillers.append(mk_rope(wi))
                    return fillers

                def run_fillers(fl, n):
                    for _ in range(n):
                        if fl:
                            fl.pop(0)()

                # ---- projections for pair 0 (plus V) run up front ----
                run_fillers_q0 = qk_fillers(0)
                run_fillers(run_fillers_q0, len(run_fillers_q0))

                wv_sb = [WV.tile([128, DG], bf16, tag="wv", name=f"wv_{_}") for _ in range(CC)]
                for i in range(CC):
                    nc.sync.dma_start(wv_sb[i][:], wv_d.ap()[i * 128:(i + 1) * 128, :])
                for sc in range(SC):
                    ssl = slice(sc * 128, (sc + 1) * 128)
                    ps = PS.tile([128, 512], f32, tag="proj", name="psv")
                    for cc in range(CC):
                        nc.tensor.matmul(
                            ps[:], xt[cc][:, ssl], wv_sb[cc][:],
                            start=(cc == 0), stop=False,
                        )
                    nc.tensor.matmul(
                        ps[:], ones[0:1, 0:128], bv_sb[:], start=False, stop=True,
                    )
                    va3 = vaug[sc][:].rearrange("p (h c) -> p h c", c=65)
                    ps3 = ps[:].rearrange("p (h c) -> p h c", c=64)
                    nc.vector.tensor_copy(va3[:, :, 0:64], ps3[:, :, :])
                    nc.vector.memset(va3[:, :, 64:65], 1.0)

                # ---- output projection + softmax normalization pieces,
                # emitted as fillers inside the last pair's attention ----
                wo_sb = [P.tile([128, D], bf16, tag=f"wo{i}", name=f"wo{i}") for i in range(DC)]
                for i in range(DC):
                    nc.sync.dma_start(wo_sb[i][:], wo_d.ap()[i * 128:(i + 1) * 128, :])
                bo_sb = P.tile([1, D], bf16, tag="bo")
                nc.sync.dma_start(bo_sb[:], bo_d.ap()[:])

                def norm_o_fillers(q):
                    qsl = slice(q * QT2, (q + 1) * QT2)
                    fl = []

                    def recip():
                        nc.vector.reciprocal_approx_fast(densbr[:, qsl], densb[:, qsl])
                        nc.vector.tensor_copy(densbb[:, qsl], densbr[:, qsl])
                    fl.append(recip)

                    def mk_norm(dc):
                        def emit():
                            recip2 = T1.tile([2, QT2], bf16, tag="recip2", bufs=2)
                            nc.sync.dma_start(recip2[:], densbb[2 * dc:2 * dc + 2, qsl])
                            psb = PS.tile([128, QT2], f32, tag="proj", name="psb")
                            nc.tensor.matmul(
                                psb[:], sel[:], recip2[:], start=True, stop=True)
                            nc.vector.tensor_tensor(
                                aoh2[dc][:, qsl], aoh2[dc][:, qsl], psb[:],
                                op=ALU.mult)
                        return emit
                    for dc in range(DC):
                        fl.append(mk_norm(dc))

                    def mk_osc(sc, nt, ob_box):
                        def emit():
                            ssl = slice(sc * 128, (sc + 1) * 128)
                            nsl = slice(nt * 512, (nt + 1) * 512)
                            if nt == 0:
                                ob_box.append(T1.tile(
                                    [128, D], f32, tag="qtsb", name=f"ob{sc}", bufs=2))
                            ps = PS.tile([128, QT2], f32, tag="proj", name=f"pso3_{sc}_{nt}")
                            for hp in range(DC):
                                nc.tensor.matmul(
                                    ps[:], aoh2[hp][:, ssl], wo_sb[hp][:, nsl],
                                    start=(hp == 0), stop=False,
                                )
                            nc.tensor.matmul(
                                ps[:], ones[0:1, 0:128], bo_sb[0:1, nsl],
                                start=False, stop=True,
                            )
                            ob = ob_box[0]
                            nc.vector.tensor_copy(ob[:, nsl], ps[:])
                            if nt == 1:
                                nc.gpsimd.dma_start(out_d.ap()[ssl, :], ob[:])
                        return emit
                    for sc in range(q * 4, q * 4 + 4):
                        ob_box = []
                        fl.append(mk_osc(sc, 0, ob_box))
                        fl.append(mk_osc(sc, 1, ob_box))
                    return fl

                # ---- attention for pair dc; fillers interleave the Q/K
                # projection for pair dc+1 (dc<3) or normalization + output
                # projection for earlier q-tiles (dc=3) into the PE stream ----
                def emit_attn(dc, fillers_by_window):
                    for q in range(NQ):
                        fillers = fillers_by_window[q]
                        per_iter = max(1, (len(fillers) + SC - 1) // SC) if fillers else 0
                        qsl = slice(q * QT2, (q + 1) * QT2)
                        pso = [
                            PS.tile([65, QT2], f32, tag="psoA", name="psoA", bufs=1),
                            PS.tile([65, QT2], f32, tag="psoB", name="psoB", bufs=1),
                        ]
                        for ks in range(SC):
                            ksl = slice(ks * 128, (ks + 1) * 128)
                            pss = PS.tile([128, 2 * QT2], f32, tag="big", name="pss")
                            for half in range(2):
                                rows = slice(64 * half, 64 * half + 64)
                                nc.tensor.matmul(
                                    pss[:, half * QT2:(half + 1) * QT2],
                                    kt[dc][rows, ksl],
                                    qt[dc][rows, qsl],
                                    start=True, stop=True,
                                )
                            ptile = PT.tile([128, 2 * QT2], bf16, tag="pt", name="ptile", bufs=3)
                            nc.scalar.activation(ptile[:], pss[:], AF.Exp, scale=0.125)
                            for half in range(2):
                                lh = 2 * dc + half
                                nc.tensor.matmul(
                                    pso[half][:],
                                    vaug[ks][:, 65 * lh:65 * lh + 65],
                                    ptile[:, half * QT2:(half + 1) * QT2],
                                    start=(ks == 0), stop=(ks == SC - 1),
                                )
                            run_fillers(fillers, per_iter)
                        run_fillers(fillers, len(fillers))
                        # evacuate: unnormalized attention out (pair packed
                        # on partitions) + denominator rows. Engine ops need
                        # 32-aligned partition bases, so the den row relays
                        # through partition 64 of a scratch tile and a small
                        # SBUF->SBUF DMA (byte-addressed) lands it on
                        # densb's packed rows.
                        for half in range(2):
                            po = 64 * half
                            nc.vector.tensor_copy(
                                aoh2[dc][po:po + 64, qsl], pso[half][0:64, :])
                            scr = T1.tile([65, QT2], f32, tag="scr", bufs=2)
                            nc.vector.tensor_copy(scr[64:65, :], pso[half][64:65, :])
                            nc.sync.dma_start(
                                densb[2 * dc + half:2 * dc + half + 1, qsl],
                                scr[64:65, :])

                for dc in range(DC):
                    if dc + 1 < DC:
                        fillers = qk_fillers(dc + 1)
                        nw = len(fillers) // NQ
                        by_window = [fillers[w * nw:(w + 1) * nw] for w in range(NQ - 1)]
                        by_window.append(fillers[(NQ - 1) * nw:])
                    else:
                        # window q runs normalization + output projection for
                        # the q-1 query range (ready since window q-1 ended)
                        by_window = [[]] + [norm_o_fillers(q) for q in range(NQ - 1)]
                    emit_attn(dc, by_window)

                # tail: normalization + output projection for the last range
                tail = norm_o_fillers(NQ - 1)
                run_fillers(tail, len(tail))

    nc.compile()
    return nc


# head-local dim permutation: evens first, odds second. Q/K projection
# columns, their biases, and the rope tables all use this layout so the
# rotate-half partner of row j is row j+-32 (a contiguous block swap).
PERM64 = np.concatenate([np.arange(0, HDIM, 2), np.arange(1, HDIM, 2)])
PERMDG = np.concatenate([h * HDIM + PERM64 for h in range(HPC)])


def _rope_tables(start):
    inv_freq = 1.0 / (ROPE_BASE ** (np.arange(0, HDIM, 2, dtype=np.float64) / HDIM))
    j = np.arange(128) % HDIM
    row_freq = inv_freq[j % 32]  # [128] permuted-row frequency
    pos = np.arange(S, dtype=np.float64)
    rel = np.where(pos >= start, pos - start, 0.0)
    ang = row_freq[:, None] * rel[None, :]
    on = (pos >= start)[None, :]
    cosT = np.where(on, np.cos(ang), 1.0)
    sinT = np.where(on, np.sin(ang), 0.0)
    # evens block (j<32) pairs with +32 partner using -sin; odds block +sin
    sign = np.where(j < 32, -1.0, 1.0)
    sinTs = sinT * sign[:, None]
    return cosT, sinTs


def prepare_in_maps(inputs):
    import ml_dtypes

    bf16 = ml_dtypes.bfloat16
    x = np.asarray(inputs["x"], dtype=np.float32)
    start = int(np.asarray(inputs["rope_start_index"]))

    cosT, sinTs = _rope_tables(start)
    cosT = cosT.astype(bf16)
    sinTs = sinTs.astype(bf16)

    xTs = [np.ascontiguousarray(x[b].T).astype(bf16) for b in range(B)]

    per_hg = []
    for hg in range(HG):
        csl = slice(hg * DG, (hg + 1) * DG)
        m = {}
        for name in ("q", "k"):
            w = np.asarray(inputs["W" + name], dtype=np.float32)[:, csl][:, PERMDG]
            bvec = np.asarray(inputs["b" + name], dtype=np.float32)[csl][PERMDG]
            m["w" + name] = np.ascontiguousarray(w).astype(bf16)
            m["b" + name + "4"] = np.ascontiguousarray(
                bvec.reshape(4, 128).T
            ).astype(np.float32)
        m["wv"] = np.asarray(inputs["Wv"], dtype=np.float32)[:, csl].astype(bf16)
        m["bv"] = np.asarray(inputs["bv"], dtype=np.float32)[None, csl].astype(bf16)
        m["wo"] = np.asarray(inputs["Wo"], dtype=np.float32)[csl, :].astype(bf16)
        bo = np.asarray(inputs["bo"], dtype=np.float32)
        m["bo"] = (bo if hg == 0 else np.zeros_like(bo))[None, :].astype(bf16)
        per_hg.append(m)

    in_maps = []
    for c in range(NCORES):
        b, hg = c // HG, c % HG
        m = per_hg[hg]
        _nw = (int(os.environ.get("BUILD_REPEAT", "1"))
               + 100 * int(os.environ.get("BUILD_NONCE", "0")))
        selm = np.zeros((2, 128), bf16)
        selm[0, 0:64] = 1.0
        selm[1, 64:128] = 1.0
        in_maps.append({
            "nonce": np.zeros((1, _nw), np.float32),
            "sel": selm,
            "xT": xTs[b],
            "wq": m["wq"], "wk": m["wk"], "wv": m["wv"], "wo": m["wo"],
            "cosT": cosT, "sinTs": sinTs,
            "bq4": m["bq4"], "bk4": m["bk4"],
            "bv": m["bv"], "bo": m["bo"],
        })
    return in_maps


def kernel(**inputs):
    from concourse.bass_utils import run_bass_kernel_spmd

    if "nc" not in _CACHE:
        _CACHE["nc"] = _build_program()
    nc = _CACHE["nc"]

    in_maps = prepare_in_maps(inputs)
    res = run_bass_kernel_spmd(nc, in_maps, core_ids=list(range(NCORES)))
    out = np.empty((B, S, D), dtype=np.float32)
    for b in range(B):
        out[b] = res.results[HG * b]["out"] + res.results[HG * b + 1]["out"]
    return out


# revision 28
# speedup vs baseline: 1.1948x; 1.1948x over previous
"""AxialRoPE self-attention on 8 Trainium2 NeuronCores.

Sharding: 8 cores = 4 batches x 2 head-groups (8 heads each).
Each core computes q/k/v projections for its head-group over the full
sequence of its batch, RoPE, attention, and a partial output projection
(row-sharded Wo). Host sums the two partial outputs per batch.

Per-core kernel (all matmuls bf16 with fp32 PSUM accumulation):
  xT [1024, 2048] = x[b]^T (host-prepped bf16)
  KT/QT = W^T x^T + b   [512, 2048] head-dim-major (K emitted first so
  attention unblocks sooner).
  RoPE: qt' = QT*cosT + shift(QT)*sinTs where shift swaps partition pairs
  (2i <-> 2i+1) via two partition-strided SBUF DMAs and sinTs carries the
  (-1)^(d+1) sign.
  scoresT[ks, qs]: the two heads of a pair land in adjacent row groups
  (lhsT base partitions 0/64) so their matmuls overlap in the PE array;
  both halves fill one [128, 1024] psum tile -> single exp (scale=1/8,
  no max subtraction: scores ~N(0,1), max < 7).
  PV: lhsT = V_aug [ks, 65] (65th column of ones -> row 64 = softmax
  denominator). Attention outputs accumulate UNNORMALIZED into
  aoh2[hp] [128, 2048] (head pair packed on partitions); denominators
  collect into densb [8, 2048].
  While attention for pair dc runs (ACT-bound), the Q/K projection
  matmuls for pair dc+1 are interleaved into the instruction stream as
  fillers for the PE gaps.
  Normalization is deferred and ACT-free (keeps the exp activation
  table resident the whole run): per q-range, reciprocal_approx_fast
  over the densb rows on the DVE, then per (hp, q-tile) a K=2 selector
  matmul broadcasts the two reciprocal rows across the 128 partitions
  and one DVE multiply rescales aoh2 in place. The normalization and
  the output projection (lhsT = aoh2[hp] [128, 128], K=128, both heads
  of the pair in one matmul) for query range q are interleaved as PE
  fillers into the q+1 attention window of the last head pair, so only
  the final q-range's epilogue runs after attention.
"""

import os
import numpy as np

B, S, D = 4, 2048, 1024
NHEAD, HDIM = 16, 64
HG = 2                # head-group shards
HPC = NHEAD // HG     # 8 heads per core
DG = HPC * HDIM       # 512 local projection width
NCORES = 8
ROPE_BASE = 10000.0

_CACHE = {}


def _build_program():
    from concourse import bass, bacc, tile
    from concourse import mybir

    dt = mybir.dt
    f32, bf16 = dt.float32, dt.bfloat16
    AF = mybir.ActivationFunctionType
    ALU = mybir.AluOpType
    PSUM = bass.MemorySpace.PSUM

    nc = bacc.Bacc("TRN2", target_bir_lowering=False, debug=False)

    # The PJRT-side NEFF cache keys on the HLO signature, which sees only
    # tensor shapes -- encode a build nonce in a dummy input's shape so
    # program variants with identical I/O still recompile.
    _nw = (int(os.environ.get("BUILD_REPEAT", "1"))
           + 100 * int(os.environ.get("BUILD_NONCE", "0")))
    nc.dram_tensor("nonce", [1, _nw], f32, kind="ExternalInput")

    xT_d = nc.dram_tensor("xT", [D, S], bf16, kind="ExternalInput")
    wq_d = nc.dram_tensor("wq", [D, DG], bf16, kind="ExternalInput")
    wk_d = nc.dram_tensor("wk", [D, DG], bf16, kind="ExternalInput")
    wv_d = nc.dram_tensor("wv", [D, DG], bf16, kind="ExternalInput")
    wo_d = nc.dram_tensor("wo", [DG, D], bf16, kind="ExternalInput")
    cos_d = nc.dram_tensor("cosT", [128, S], bf16, kind="ExternalInput")
    sin_d = nc.dram_tensor("sinTs", [128, S], bf16, kind="ExternalInput")
    bq_d = nc.dram_tensor("bq4", [128, 4], f32, kind="ExternalInput")
    bk_d = nc.dram_tensor("bk4", [128, 4], f32, kind="ExternalInput")
    bv_d = nc.dram_tensor("bv", [1, DG], bf16, kind="ExternalInput")
    bo_d = nc.dram_tensor("bo", [1, D], bf16, kind="ExternalInput")
    sel_d = nc.dram_tensor("sel", [2, 128], bf16, kind="ExternalInput")
    out_d = nc.dram_tensor("out", [S, D], f32, kind="ExternalOutput")

    CC = D // 128    # 8 contraction chunks
    DC = DG // 128   # 4 dout chunks (2 heads each)
    SC = S // 128    # 16 sequence chunks
    QT2 = 512        # query tile
    NQ = S // QT2    # 4

    with tile.TileContext(nc) as tc:
        with (
            tc.tile_pool(name="persist", bufs=1) as P,
            tc.tile_pool(name="ps_all", bufs=2, space=PSUM) as PS,
            tc.tile_pool(name="tmp", bufs=2) as T1,
            tc.tile_pool(name="wstream", bufs=16) as WS,
            tc.tile_pool(name="wvp", bufs=8) as WV,
            tc.tile_pool(name="ptp", bufs=2) as PT,
        ):
            _REPEAT = int(os.environ.get("BUILD_REPEAT", "1"))
            for _rep in range(_REPEAT):
                # ---- weight DMAs first (small, unblock projections) ----
                wqk_sb = {}
                for wi, w_d in enumerate([wk_d, wq_d]):  # K first
                    wqk_sb[wi] = [WS.tile([128, DG], bf16, tag="w", name=f"w{wi}_{_}")
                                  for _ in range(CC)]
                    for i in range(CC):
                        nc.sync.dma_start(wqk_sb[wi][i][:], w_d.ap()[i * 128:(i + 1) * 128, :])
                xt = [P.tile([128, S], bf16, tag=f"xt{i}", name=f"xt{i}") for i in range(CC)]
                for i in range(CC):
                    # gpsimd queue: overlaps with the weight DMAs on sync
                    nc.gpsimd.dma_start(xt[i][:], xT_d.ap()[i * 128:(i + 1) * 128, :])
                cos_t = P.tile([128, S], bf16, tag="cos")
                sin_t = P.tile([128, S], bf16, tag="sin")
                nc.sync.dma_start(cos_t[:], cos_d.ap()[:])
                nc.sync.dma_start(sin_t[:], sin_d.ap()[:])
                bq4 = P.tile([128, 4], f32, tag="bq4")
                bk4 = P.tile([128, 4], f32, tag="bk4")
                bv_sb = P.tile([1, DG], bf16, tag="bv_sb")
                nc.sync.dma_start(bq4[:], bq_d.ap()[:])
                nc.sync.dma_start(bk4[:], bk_d.ap()[:])
                nc.sync.dma_start(bv_sb[:], bv_d.ap()[:])

                ones = P.tile([1, 128], bf16, tag="ones")
                nc.vector.memset(ones[:], 1.0)
                # selector for denominator broadcast: out row j gets
                # recip row 0 (j<64) or row 1 (j>=64)
                sel = P.tile([2, 128], bf16, tag="sel")
                nc.sync.dma_start(sel[:], sel_d.ap()[:])

                qt = [P.tile([128, S], bf16, tag=f"qt{i}", name=f"qt{i}") for i in range(DC)]
                kt = [P.tile([128, S], bf16, tag=f"kt{i}", name=f"kt{i}") for i in range(DC)]
                vaug = [P.tile([128, HPC * 65], bf16, tag=f"va{i}", name=f"va{i}") for i in range(SC)]
                aoh2 = [P.tile([128, S], bf16, tag=f"ao{i}", name=f"ao{i}") for i in range(DC)]
                densb = P.tile([8, S], f32, tag="densb")
                densbr = P.tile([8, S], f32, tag="densbr")
                densbb = P.tile([8, S], bf16, tag="densbb")

                # ---- Q/K projection + rope for pair dc, as a list of
                # filler closures (each emits ~1 PE matmul or the DVE/DMA
                # epilogue for one 512-col s-tile) ----
                def qk_fillers(dc):
                    dsl = slice(dc * 128, (dc + 1) * 128)
                    fillers = []
                    state = {}

                    def mk_mm(wi, st, cc):
                        def emit():
                            sl = slice(st * 512, (st + 1) * 512)
                            if cc == 0:
                                state[(wi, st)] = PS.tile(
                                    [128, 512], f32, tag="proj", name=f"psp{dc}_{wi}_{st}")
                            ps = state[(wi, st)]
                            nc.tensor.matmul(
                                ps[:], wqk_sb[wi][cc][:, dsl], xt[cc][:, sl],
                                start=(cc == 0), stop=(cc == CC - 1),
                            )
                        return emit

                    def mk_bias(wi, st):
                        def emit():
                            sl = slice(st * 512, (st + 1) * 512)
                            b4 = bk4 if wi == 0 else bq4
                            qtsb = state[("sb", wi)]
                            nc.vector.tensor_scalar(
                                qtsb[:, sl], state[(wi, st)][:], b4[:, dc:dc + 1],
                                None, op0=ALU.add,
                            )
                        return emit

                    def mk_sb(wi):
                        def emit():
                            state[("sb", wi)] = T1.tile(
                                [128, S], bf16, tag="qtsb", bufs=2, name=f"qtsb{dc}_{wi}")
                        return emit

                    def mk_rope(wi):
                        def emit():
                            dst = (kt if wi == 0 else qt)[dc]
                            qtsb = state[("sb", wi)]
                            qsh = T1.tile([128, S], bf16, tag="qsh", bufs=1)
                            for blk in range(2):
                                b0 = 64 * blk
                                nc.sync.dma_start(
                                    qsh[b0:b0 + 32, :], qtsb[b0 + 32:b0 + 64, :])
                                nc.sync.dma_start(
                                    qsh[b0 + 32:b0 + 64, :], qtsb[b0:b0 + 32, :])
                            nc.vector.tensor_tensor(dst[:], qtsb[:], cos_t[:], op=ALU.mult)
                            tt2 = T1.tile([128, S], bf16, tag="tt2", bufs=1)
                            nc.vector.tensor_tensor(tt2[:], qsh[:], sin_t[:], op=ALU.mult)
                            nc.vector.tensor_tensor(dst[:], dst[:], tt2[:], op=ALU.add)
                        return emit

                    for wi in range(2):  # 0=K, 1=Q
                        fillers.append(mk_sb(wi))
                        for st in range(4):
                            for cc in range(CC):
                                fillers.append(mk_mm(wi, st, cc))
                            fillers.append(mk_bias(wi, st))
                        fillers.append(mk_rope(wi))
                    return fillers

                def run_fillers(fl, n):
                    for _ in range(n):
                        if fl:
                            fl.pop(0)()

                # ---- projections for pair 0 (plus V) run up front ----
                run_fillers_q0 = qk_fillers(0)
                run_fillers(run_fillers_q0, len(run_fillers_q0))

                wv_sb = [WV.tile([128, DG], bf16, tag="wv", name=f"wv_{_}") for _ in range(CC)]
                for i in range(CC):
                    nc.sync.dma_start(wv_sb[i][:], wv_d.ap()[i * 128:(i + 1) * 128, :])
                for sc in range(SC):
                    ssl = slice(sc * 128, (sc + 1) * 128)
                    ps = PS.tile([128, 512], f32, tag="proj", name="psv")
                    for cc in range(CC):
                        nc.tensor.matmul(
                            ps[:], xt[cc][:, ssl], wv_sb[cc][:],
                            start=(cc == 0), stop=False,
                        )
                    nc.tensor.matmul(
                        ps[:], ones[0:1, 0:128], bv_sb[:], start=False, stop=True,
                    )
                    va3 = vaug[sc][:].rearrange("p (h c) -> p h c", c=65)
                    ps3 = ps[:].rearrange("p (h c) -> p h c", c=64)
                    nc.vector.tensor_copy(va3[:, :, 0:64], ps3[:, :, :])
                    nc.vector.memset(va3[:, :, 64:65], 1.0)

                # ---- output projection + softmax normalization pieces,
                # emitted as fillers inside the last pair's attention ----
                wo_sb = [P.tile([128, D], bf16, tag=f"wo{i}", name=f"wo{i}") for i in range(DC)]
                for i in range(DC):
                    nc.sync.dma_start(wo_sb[i][:], wo_d.ap()[i * 128:(i + 1) * 128, :])
                bo_sb = P.tile([1, D], bf16, tag="bo")
                nc.sync.dma_start(bo_sb[:], bo_d.ap()[:])

                def norm_o_fillers(q):
                    qsl = slice(q * QT2, (q + 1) * QT2)
                    fl = []

                    def recip():
                        nc.vector.reciprocal_approx_fast(densbr[:, qsl], densb[:, qsl])
                        nc.vector.tensor_copy(densbb[:, qsl], densbr[:, qsl])
                    fl.append(recip)

                    def mk_norm(dc):
                        def emit():
                            recip2 = T1.tile([2, QT2], bf16, tag="recip2", bufs=2)
                            nc.sync.dma_start(recip2[:], densbb[2 * dc:2 * dc + 2, qsl])
                            psb = PS.tile([128, QT2], f32, tag="proj", name="psb")
                            nc.tensor.matmul(
                                psb[:], sel[:], recip2[:], start=True, stop=True)
                            nc.vector.tensor_tensor(
                                aoh2[dc][:, qsl], aoh2[dc][:, qsl], psb[:],
                                op=ALU.mult)
                        return emit
                    for dc in range(DC):
                        fl.append(mk_norm(dc))

                    def mk_osc(sc, nt, ob_box):
                        def emit():
                            ssl = slice(sc * 128, (sc + 1) * 128)
                            nsl = slice(nt * 512, (nt + 1) * 512)
                            if nt == 0:
                                ob_box.append(T1.tile(
                                    [128, D], f32, tag="qtsb", name=f"ob{sc}", bufs=2))
                            ps = PS.tile([128, QT2], f32, tag="proj", name=f"pso3_{sc}_{nt}")
                            for hp in range(DC):
                                nc.tensor.matmul(
                                    ps[:], aoh2[hp][:, ssl], wo_sb[hp][:, nsl],
                                    start=(hp == 0), stop=False,
                                )
                            nc.tensor.matmul(
                                ps[:], ones[0:1, 0:128], bo_sb[0:1, nsl],
                                start=False, stop=True,
                            )
                            ob = ob_box[0]
                            nc.vector.tensor_copy(ob[:, nsl], ps[:])
                            if nt == 1:
                                nc.gpsimd.dma_start(out_d.ap()[ssl, :], ob[:])
                        return emit
                    for sc in range(q * 4, q * 4 + 4):
                        ob_box = []
                        fl.append(mk_osc(sc, 0, ob_box))
                        fl.append(mk_osc(sc, 1, ob_box))
                    return fl

                # ---- attention for pair dc; fillers interleave the Q/K
                # projection for pair dc+1 (dc<3) or normalization + output
                # projection for earlier q-tiles (dc=3) into the PE stream ----
                def emit_attn(dc, fillers_by_window):
                    for q in range(NQ):
                        fillers = fillers_by_window[q]
                        per_iter = max(1, (len(fillers) + SC - 1) // SC) if fillers else 0
                        qsl = slice(q * QT2, (q + 1) * QT2)
                        pso = [
                            PS.tile([65, QT2], f32, tag="psoA", name="psoA", bufs=1),
                            PS.tile([65, QT2], f32, tag="psoB", name="psoB", bufs=1),
                        ]
                        for ks in range(SC):
                            ksl = slice(ks * 128, (ks + 1) * 128)
                            pss = PS.tile([128, 2 * QT2], f32, tag="big", name="pss")
                            for half in range(2):
                                rows = slice(64 * half, 64 * half + 64)
                                nc.tensor.matmul(
                                    pss[:, half * QT2:(half + 1) * QT2],
                                    kt[dc][rows, ksl],
                                    qt[dc][rows, qsl],
                                    start=True, stop=True,
                                )
                            ptile = PT.tile([128, 2 * QT2], bf16, tag="pt", name="ptile", bufs=3)
                            nc.scalar.activation(ptile[:], pss[:], AF.Exp, scale=0.125)
                            for half in range(2):
                                lh = 2 * dc + half
                                nc.tensor.matmul(
                                    pso[half][:],
                                    vaug[ks][:, 65 * lh:65 * lh + 65],
                                    ptile[:, half * QT2:(half + 1) * QT2],
                                    start=(ks == 0), stop=(ks == SC - 1),
                                )
                            run_fillers(fillers, per_iter)
                        run_fillers(fillers, len(fillers))
                        # evacuate: unnormalized attention out (pair packed
                        # on partitions) + denominator rows. Engine ops need
                        # 32-aligned partition bases, so the den row relays
                        # through partition 64 of a scratch tile and a small
                        # SBUF->SBUF DMA (byte-addressed) lands it on
                        # densb's packed rows.
                        for half in range(2):
                            po = 64 * half
                            nc.vector.tensor_copy(
                                aoh2[dc][po:po + 64, qsl], pso[half][0:64, :])
                            scr = T1.tile([65, QT2], f32, tag="scr", bufs=2)
                            nc.vector.tensor_copy(scr[64:65, :], pso[half][64:65, :])
                            nc.sync.dma_start(
                                densb[2 * dc + half:2 * dc + half + 1, qsl],
                                scr[64:65, :])

                for dc in range(DC):
                    if dc + 1 < DC:
                        fillers = qk_fillers(dc + 1)
                        nw = len(fillers) // NQ
                        by_window = [fillers[w * nw:(w + 1) * nw] for w in range(NQ - 1)]
                        by_window.append(fillers[(NQ - 1) * nw:])
                    else:
                        # window q runs normalization + output projection for
                        # the q-1 query range (ready since window q-1 ended)
                        by_window = [[]] + [norm_o_fillers(q) for q in range(NQ - 1)]
                    emit_attn(dc, by_window)

                # tail: normalization + output projection for the last range
                tail = norm_o_fillers(NQ - 1)
                run_fillers(tail, len(tail))

    nc.compile()
    return nc


# head-local dim permutation: evens first, odds second. Q/K projection
# columns, their biases, and the rope tables all use this layout so the
# rotate-half partner of row j is row j+-32 (a contiguous block swap).
PERM64 = np.concatenate([np.arange(0, HDIM, 2), np.arange(1, HDIM, 2)])
PERMDG = np.concatenate([h * HDIM + PERM64 for h in range(HPC)])


def _rope_tables(start):
    inv_freq = 1.0 / (ROPE_BASE ** (np.arange(0, HDIM, 2, dtype=np.float64) / HDIM))
    j = np.arange(128) % HDIM
    row_freq = inv_freq[j % 32]  # [128] permuted-row frequency
    pos = np.arange(S, dtype=np.float64)
    rel = np.where(pos >= start, pos - start, 0.0)
    ang = row_freq[:, None] * rel[None, :]
    on = (pos >= start)[None, :]
    cosT = np.where(on, np.cos(ang), 1.0)
    sinT = np.where(on, np.sin(ang), 0.0)
    # evens block (j<32) pairs with +32 partner using -sin; odds block +sin
    sign = np.where(j < 32, -1.0, 1.0)
    sinTs = sinT * sign[:, None]
    return cosT, sinTs


def prepare_in_maps(inputs):
    import ml_dtypes

    bf16 = ml_dtypes.bfloat16
    x = np.asarray(inputs["x"], dtype=np.float32)
    start = int(np.asarray(inputs["rope_start_index"]))

    cosT, sinTs = _rope_tables(start)
    cosT = cosT.astype(bf16)
    sinTs = sinTs.astype(bf16)

    xTs = [np.ascontiguousarray(x[b].T).astype(bf16) for b in range(B)]

    per_hg = []
    for hg in range(HG):
        csl = slice(hg * DG, (hg + 1) * DG)
        m = {}
        for name in ("q", "k"):
            w = np.asarray(inputs["W" + name], dtype=np.float32)[:, csl][:, PERMDG]
            bvec = np.asarray(inputs["b" + name], dtype=np.float32)[csl][PERMDG]
            m["w" + name] = np.ascontiguousarray(w).astype(bf16)
            m["b" + name + "4"] = np.ascontiguousarray(
                bvec.reshape(4, 128).T
            ).astype(np.float32)
        m["wv"] = np.asarray(inputs["Wv"], dtype=np.float32)[:, csl].astype(bf16)
        m["bv"] = np.asarray(inputs["bv"], dtype=np.float32)[None, csl].astype(bf16)
        m["wo"] = np.asarray(inputs["Wo"], dtype=np.float32)[csl, :].astype(bf16)
        bo = np.asarray(inputs["bo"], dtype=np.float32)
        m["bo"] = (bo if hg == 0 else np.zeros_like(bo))[None, :].astype(bf16)
        per_hg.append(m)

    in_maps = []
    for c in range(NCORES):
        b, hg = c // HG, c % HG
        m = per_hg[hg]
        _nw = (int(os.environ.get("BUILD_REPEAT", "1"))
               + 100 * int(os.environ.get("BUILD_NONCE", "0")))
        selm = np.zeros((2, 128), bf16)
        selm[0, 0:64] = 1.0
        selm[1, 64:128] = 1.0
        in_maps.append({
            "nonce": np.zeros((1, _nw), np.float32),
            "sel": selm,
            "xT": xTs[b],
            "wq": m["wq"], "wk": m["wk"], "wv": m["wv"], "wo": m["wo"],
            "cosT": cosT, "sinTs": sinTs,
            "bq4": m["bq4"], "bk4": m["bk4"],
            "bv": m["bv"], "bo": m["bo"],
        })
    return in_maps


def kernel(**inputs):
    from concourse.bass_utils import run_bass_kernel_spmd

    if "nc" not in _CACHE:
        _CACHE["nc"] = _build_program()
    nc = _CACHE["nc"]

    in_maps = prepare_in_maps(inputs)
    res = run_bass_kernel_spmd(nc, in_maps, core_ids=list(range(NCORES)))
    out = np.empty((B, S, D), dtype=np.float32)
    for b in range(B):
        out[b] = res.results[HG * b]["out"] + res.results[HG * b + 1]["out"]
    return out
